# revision 4
# baseline (speedup 1.0000x reference)
"""Trainium2 Bass kernel for nn_LASCC (sparse patch-correlation attention + top-k).

Math (per batch element b):
  x_hat = L2-normalize(x, dim=channels)
  z_p[c, n] = x_hat at the two in-patch diagonal pixels (p=0: (0,0), p=1: (1,1))
  C_p = z_p^T z_p                  (1024x1024 normalized correlation, symmetric)
  C_2 = (C_0 + C_1)/2              (avg map)
  A_q = softmax_row * softmax_col = E^2/(r[n] r[m]),  E=exp(alpha*mask*C), r=rowsum(E)
  out pixel with patch n, map q: top-3 over m of A_q[n, m]
    = (top-3 over m of E[n,m]*sqrt(u[m]))^2 * u[n],  u = 1/r

v3 design notes:
  - E stored once as fp16 slab (exp with rowsum accumulation); F-phase is
    v = E * bcast(1/sqrt(r)) (DVE 2x fp16) + MAX8.
  - ALL reciprocal-sqrts via bit-hack + 2 Newton steps (vector int init,
    gpsimd float iterations) -- no Ln/Sqrt activations, so the scalar engine
    keeps one ACT table set for the entire kernel (exp; square is a filler
    present in every set). v2 lost ~22us to table-set thrash.
  - channel norms computed in [128, 8] transposed layout via 8 small matmuls
    per chain; inverse norms broadcast by DMA through DRAM (partition
    broadcast), not PSUM matmul broadcast.
  - avg-map slab s2 = s0+s1 on gpsimd, interleaved per chunk with the q0/q1
    E-phase; its exps lag one chunk. Removes 1/3 of big matmuls+mask mults.
  - final (t3^2)*u[n] batched: two small DVE tensor_tensor with a stride-0
    broadcast AP per stage instead of 8 scalar_tensor_tensor.
  - u/inv DMAs issued from the gpsimd queue (same engine as the chain tail).
Stages software-pipelined: F(b0,q) emission interleaves with E-chunks of b1.
"""
import numpy as np

import concourse.bass as bass
import concourse.mybir as mybir
from concourse import bacc
from concourse.tile import TileContext
from concourse.bass_utils import run_bass_kernel_spmd

F32 = mybir.dt.float32
F32R = mybir.dt.float32r
F16 = mybir.dt.float16
I32 = mybir.dt.int32
AF = mybir.ActivationFunctionType
ALU = mybir.AluOpType

B_FULL = 16
N_CORES = 8
B_LOC = B_FULL // N_CORES  # 2
C = 128
H = W = 64
NPH = 32
NP = 1024  # patches
PS = 2
TOPK = 3
NCHUNK = NP // 128  # 8

LAST_EXEC_NS = None


def _build_mask() -> np.ndarray:
    """(1 - gaussian) self-suppression mask, [NP, NP] (matches reference)."""
    rat_s = np.float32(0.05)
    sr = np.float32(NPH) * rat_s
    ind_r = np.arange(NPH, dtype=np.float32).reshape(1, NPH, 1)
    ind_c = np.arange(NPH, dtype=np.float32).reshape(1, 1, NPH)
    cent = np.arange(NPH, dtype=np.float32)
    cent_r = np.repeat(cent, NPH).reshape(NP, 1, 1)
    cent_c = np.tile(cent, NPH).reshape(NP, 1, 1)
    g = np.exp(-((ind_r - cent_r) ** 2) / (2.0 * sr * sr)) * np.exp(
        -((ind_c - cent_c) ** 2) / (2.0 * sr * sr)
    )
    return (1.0 - g).reshape(NP, NP)


def build_nc():
    nc = bacc.Bacc(trn_type="TRN2")

    x_d = nc.dram_tensor("x", [B_LOC, C, H * W], F32, kind="ExternalInput")
    mask_d = nc.dram_tensor("mask", [NP, NP], F16, kind="ExternalInput")
    alpha_d = nc.dram_tensor("alpha", [128, 1], F32, kind="ExternalInput")
    out_d = nc.dram_tensor("out", [B_LOC, 3, NP, TOPK], F32, kind="ExternalOutput")

    with TileContext(nc) as tc:
        with tc.tile_pool(name="const", bufs=1) as cpool, \
             tc.tile_pool(name="slab", bufs=4) as slabp, \
             tc.tile_pool(name="eslab", bufs=4) as epool, \
             tc.tile_pool(name="z", bufs=1) as zpool, \
             tc.tile_pool(name="work", bufs=3) as work, \
             tc.tile_pool(name="small", bufs=3) as small, \
             tc.tile_pool(name="ps", bufs=3, space="PSUM") as ps, \
             tc.tile_pool(name="psn", bufs=2, space="PSUM") as psn, \
             tc.tile_pool(name="dsc", bufs=4, space="DRAM") as dsc:

            # ---- constants
            mask_sb = cpool.tile([128, NCHUNK, NP], F16)  # mask[128i+p, m] at [p,i,m]
            nc.sync.dma_start(
                mask_sb, mask_d[:, :].rearrange("(i p) m -> p i m", p=128)
            )
            ones_k = cpool.tile([128, 1], F32)  # column-sum matmul lhsT
            nc.vector.memset(ones_k, 1.0)
            av = cpool.tile([128, 1], F32)  # alpha
            nc.sync.dma_start(av, alpha_d[:, :])
            av_h = cpool.tile([128, 1], F32)  # alpha/2
            nc.vector.tensor_scalar_mul(av_h, av, 0.5)
            scale_E = [av, av, av_h]
            c15 = cpool.tile([128, 8], F32)
            nc.vector.memset(c15, 1.5)
            chalf = cpool.tile([128, 8], F32)
            nc.vector.memset(chalf, 0.5)
            cone_i = cpool.tile([128, 8], I32)
            nc.vector.memset(cone_i, 1)
            cmagic = cpool.tile([128, 8], I32)
            nc.vector.memset(cmagic, 0x5F3759DF)

            def emit_rsqrt(src, out_dtype, tag):
                """1/sqrt(src) elementwise on [128,8]: DVE bit-hack seed +
                2 gpsimd Newton iterations. ~5e-6 rel err."""
                yi = small.tile([128, 8], I32, name="yi", tag=f"yi{tag}")
                nc.vector.tensor_tensor(out=yi, in0=src.bitcast(I32),
                                        in1=cone_i, op=ALU.logical_shift_right)
                y0i = small.tile([128, 8], I32, name="y0i", tag=f"y0{tag}")
                nc.vector.tensor_tensor(out=y0i, in0=cmagic, in1=yi,
                                        op=ALU.subtract)
                cur = y0i.bitcast(F32)
                for it in range(2):
                    t1 = small.tile([128, 8], F32, name="t1", tag=f"t1{tag}")
                    nc.gpsimd.tensor_tensor(out=t1, in0=cur, in1=cur,
                                            op=ALU.mult)
                    t2 = small.tile([128, 8], F32, name="t2", tag=f"t2{tag}")
                    nc.gpsimd.tensor_tensor(out=t2, in0=t1, in1=src,
                                            op=ALU.mult)
                    t2h = small.tile([128, 8], F32, name="t2h", tag=f"t2h{tag}")
                    nc.gpsimd.tensor_tensor(out=t2h, in0=t2, in1=chalf,
                                            op=ALU.mult)
                    t3 = small.tile([128, 8], F32, name="t3", tag=f"t3{tag}")
                    nc.gpsimd.tensor_tensor(out=t3, in0=c15, in1=t2h,
                                            op=ALU.subtract)
                    odt = out_dtype if it == 1 else F32
                    t4 = small.tile([128, 8], odt, name="t4", tag=f"t4{tag}{it}")
                    nc.gpsimd.tensor_tensor(out=t4, in0=cur, in1=t3,
                                            op=ALU.mult)
                    cur = t4
                return cur

            # ---- phase N: channel norms + normalized z (per batch element).
            # Norms land transposed [128, 8] so the rsqrt chain and the DMA
            # partition-broadcast replace Ln/Exp activations + PSUM broadcast.
            zp = {}

            def emit_norm(b):
                xs = slabp.tile([128, H * W], F32, name=f"xs{b}", tag="slab16")
                nc.sync.dma_start(xs, x_d[b])
                xr = xs.rearrange("c (i r j s) -> c r s i j", r=PS, s=PS, j=NPH)
                for p in range(PS):
                    zv = xr[:, p, p]
                    zsq = work.tile([128, NP], F32, name="zsq", tag="zsq",
                                    bufs=2)
                    nc.scalar.activation(
                        zsq.rearrange("c (a b) -> c a b", a=NPH), zv, AF.Square)
                    nrmT = psn.tile([128, NCHUNK], F32, name="nrmT", tag="nrmT")
                    for i in range(NCHUNK):
                        nc.tensor.matmul(nrmT[:, i:i + 1],
                                         zsq[:, 128 * i:128 * (i + 1)],
                                         ones_k, start=True, stop=True)
                    rTn = small.tile([128, NCHUNK], F32, name="rTn", tag="rTn")
                    nc.vector.tensor_copy(rTn, nrmT)
                    inv = emit_rsqrt(rTn, F32, "n")
                    inv_dram = dsc.tile([NP], F32, name="inv_dram",
                                        tag="inv_dram")
                    nc.gpsimd.dma_start(
                        inv_dram[:].rearrange("(i p) -> p i", p=128), inv)
                    ibc = work.tile([128, NP], F32, name="ibc", tag="ibc",
                                    bufs=2)
                    nc.gpsimd.dma_start(
                        ibc,
                        inv_dram[:].rearrange("(a m) -> a m", a=1)
                        .partition_broadcast(128))
                    z = zpool.tile([128, NP], F32R, name=f"z{b}{p}",
                                   tag=f"z{b}{p}", bufs=1)
                    nc.vector.tensor_tensor(
                        out=z.rearrange("c (a b) -> c a b", a=NPH), in0=zv,
                        in1=ibc.rearrange("c (a b) -> c a b", a=NPH),
                        op=ALU.mult)
                    zp[(b, p)] = z

            # ---- phase M
            def emit_E_chunks(st, lo, hi):
                """chunks [lo,hi) of the merged q0/q1/q2 E-phase of batch b."""
                b = st["b"]
                for i in range(lo, hi):
                    for q in range(2):
                        G = ps.tile([128, NP], F32, name="G", tag="G")
                        z = zp[(b, q)]
                        for h in range(2):
                            nc.tensor.matmul(
                                G[:, 512 * h:512 * (h + 1)],
                                z[:, 128 * i:128 * (i + 1)],
                                z[:, 512 * h:512 * (h + 1)],
                                start=True, stop=True)
                        nc.vector.scalar_tensor_tensor(
                            out=st["s"][q][:, i, :], in0=G, scalar=1.0,
                            in1=mask_sb[:, i, :], op0=ALU.mult, op1=ALU.mult)
                        nc.scalar.activation(
                            st["e"][q][:, i, :], st["s"][q][:, i, :],
                            AF.Exp, scale=scale_E[q],
                            accum_out=st["rT"][q][:, i:i + 1])
                    s2 = work.tile([128, NP], F16, name="s2", tag="s2", bufs=2)
                    nc.gpsimd.tensor_tensor(out=s2, in0=st["s"][0][:, i, :],
                                            in1=st["s"][1][:, i, :], op=ALU.add)
                    st["s2scr"][i] = s2
                    if i > 0:
                        j = i - 1
                        nc.scalar.activation(
                            st["e"][2][:, j, :], st["s2scr"][j],
                            AF.Exp, scale=scale_E[2],
                            accum_out=st["rT"][2][:, j:j + 1])

            def emit_E_finish(st):
                nc.scalar.activation(
                    st["e"][2][:, NCHUNK - 1, :], st["s2scr"][NCHUNK - 1],
                    AF.Exp, scale=scale_E[2],
                    accum_out=st["rT"][2][:, NCHUNK - 1:NCHUNK])
                for q in range(3):
                    uT = small.tile([128, NCHUNK], F32, name="uT", tag=f"uT{q}")
                    nc.vector.reciprocal(uT, st["rT"][q])
                    st["uT"][q] = uT
                    rsq = emit_rsqrt(st["rT"][q], F16, "u")
                    u_dram = dsc.tile([NP], F16, name="u_dram", tag="u_dram")
                    nc.gpsimd.dma_start(
                        u_dram[:].rearrange("(i p) -> p i", p=128), rsq)
                    squbc = work.tile([128, NP], F16, name="squbc",
                                      tag="squbc", bufs=3)
                    nc.gpsimd.dma_start(
                        squbc,
                        u_dram[:].rearrange("(a m) -> a m", a=1)
                        .partition_broadcast(128))
                    st["squbc"][q] = squbc

            def new_stage(b):
                st = dict(b=b, s={}, e={}, rT={}, uT={}, squbc={}, s2scr={})
                for q in range(2):
                    st["s"][q] = slabp.tile([128, NCHUNK, NP], F16,
                                            name=f"s{q}", tag="slab16")
                for q in range(3):
                    st["e"][q] = epool.tile([128, NCHUNK, NP], F16,
                                            name=f"e{q}", tag="eslab")
                    st["rT"][q] = small.tile([128, NCHUNK], F32, name="rT",
                                             tag=f"rT{q}")
                return st

            def emit_F(st, q):
                """v = E*sqrt(u[m]) -> top-8 -> (t3^2)*u[n] -> store."""
                b = st["b"]
                t8s = work.tile([128, NCHUNK, 8], F16, name="t8s", tag="t8s",
                                bufs=2)
                for i in range(NCHUNK):
                    v = work.tile([128, NP], F16, name="v", tag="v")
                    nc.vector.tensor_tensor(out=v, in0=st["e"][q][:, i, :],
                                            in1=st["squbc"][q], op=ALU.mult)
                    nc.vector.max(out=t8s[:, i, :], in_=v)
                w = work.tile([128, NCHUNK, TOPK], F32, name="w", tag="w",
                              bufs=2)
                u3 = st["uT"][q].unsqueeze(-1).to_broadcast(
                    [128, NCHUNK, TOPK])
                nc.vector.tensor_tensor(out=w, in0=t8s[:, :, :TOPK], in1=u3,
                                        op=ALU.mult)
                oacc = work.tile([128, NCHUNK, TOPK], F32, name="oacc",
                                 tag="oacc", bufs=2)
                nc.vector.tensor_tensor(out=oacc, in0=w, in1=t8s[:, :, :TOPK],
                                        op=ALU.mult)
                dst = out_d[b, q].rearrange("(i p) k -> p i k", p=128)
                nc.sync.dma_start(dst, oacc)

            # ---- emission schedule (software pipeline across the 2 batches)
            emit_norm(0)
            st0 = new_stage(0)
            emit_E_chunks(st0, 0, NCHUNK)
            emit_E_finish(st0)
            emit_norm(1)
            st1 = new_stage(1)
            emit_F(st0, 0)
            emit_E_chunks(st1, 0, 4)
            emit_F(st0, 1)
            emit_E_chunks(st1, 4, NCHUNK)
            emit_E_finish(st1)
            emit_F(st0, 2)
            emit_F(st1, 0)
            emit_F(st1, 1)
            emit_F(st1, 2)

    nc.compile()
    return nc


_NC_CACHE = None


def _get_nc():
    global _NC_CACHE
    if _NC_CACHE is None:
        _NC_CACHE = build_nc()
    return _NC_CACHE


def kernel(x: np.ndarray, alpha: np.ndarray) -> np.ndarray:
    global LAST_EXEC_NS
    x = np.ascontiguousarray(np.asarray(x, dtype=np.float32))
    alpha_arr = np.full((128, 1), np.float32(np.asarray(alpha)),
                        dtype=np.float32)
    mask = _build_mask().astype(np.float16)

    nc = _get_nc()
    in_maps = []
    for core in range(N_CORES):
        xs = x[core * B_LOC:(core + 1) * B_LOC].reshape(B_LOC, C, H * W)
        in_maps.append({"x": np.ascontiguousarray(xs), "mask": mask,
                        "alpha": alpha_arr})
    res = run_bass_kernel_spmd(nc, in_maps, core_ids=list(range(N_CORES)))
    LAST_EXEC_NS = res.exec_time_ns

    # assemble: out[bg, k, 2i+dr, 2j+dc] from T_q[b, n=i*32+j, k]
    out = np.empty((B_FULL, TOPK, H, W), dtype=np.float32)
    for core in range(N_CORES):
        t = res.results[core]["out"]  # [B_LOC, 3, NP, TOPK]
        for bl in range(B_LOC):
            bg = core * B_LOC + bl
            tq = t[bl].reshape(3, NPH, NPH, TOPK).transpose(0, 3, 1, 2)
            out[bg, :, 0::2, 0::2] = tq[0]
            out[bg, :, 1::2, 1::2] = tq[1]
            out[bg, :, 0::2, 1::2] = tq[2]
            out[bg, :, 1::2, 0::2] = tq[2]
    return out


# revision 6
# speedup vs baseline: 1.2399x; 1.2399x over previous
"""Trainium2 Bass kernel for nn_LASCC (sparse patch-correlation attention + top-k).

Math (per batch element b):
  x_hat = L2-normalize(x, dim=channels)
  z_p[c, n] = x_hat at the two in-patch diagonal pixels (p=0: (0,0), p=1: (1,1))
  C_p = z_p^T z_p                  (1024x1024 normalized correlation, symmetric)
  C_2 = (C_0 + C_1)/2              (avg map)
  A_q = softmax_row * softmax_col = E^2/(r[n] r[m]),  E=exp(alpha*mask*C), r=rowsum(E)
  out pixel with patch n, map q: top-3 over m of A_q[n, m]
    = (top-3 over m of E[n,m]*sqrt(u[m]))^2 * u[n],  u = 1/r

v4 design notes (v3 post-mortem: gpsimd in-order queue serialized every stage
boundary -- rsqrt chains, DMA issues and s2-adds all stacked there; 66us of
gpsimd sem waits and 299us span):
  - E stored once as fp16 slab (exp + rowsum accumulation); F-phase is
    v = E * bcast(1/sqrt(r)) (DVE fp16 2x) + MAX8.
  - all reciprocal-sqrts via bit-hack + Newton (int seed on DVE, float
    iterations on gpsimd; 2 iters for channel norms, 1 for stage u) -- no
    Ln/Sqrt activations, so ONE ACT table set serves the whole kernel.
  - every dma_start issued from the (otherwise idle) sync engine.
  - channel norms in [128, 8] transposed layout via 8 small matmuls per
    chain; inverse norms broadcast by DMA partition-broadcast; the
    normalize multiply runs on gpsimd, off the critical DVE stream.
  - avg-map slab s2 = s0+s1 on gpsimd interleaved per chunk; its exps lag
    one chunk. Removes 1/3 of big matmuls + mask multiplies.
  - final (t3^2)*u[n] batched into two small stride-0-broadcast DVE ops.
Stages software-pipelined: F(b0,q) emission interleaves with E-chunks of b1.
"""
import numpy as np

import concourse.bass as bass
import concourse.mybir as mybir
from concourse import bacc
from concourse.tile import TileContext
from concourse.bass_utils import run_bass_kernel_spmd

F32 = mybir.dt.float32
F32R = mybir.dt.float32r
F16 = mybir.dt.float16
I32 = mybir.dt.int32
AF = mybir.ActivationFunctionType
ALU = mybir.AluOpType

B_FULL = 16
N_CORES = 8
B_LOC = B_FULL // N_CORES  # 2
C = 128
H = W = 64
NPH = 32
NP = 1024  # patches
PS = 2
TOPK = 3
NCHUNK = NP // 128  # 8

LAST_EXEC_NS = None


def _build_mask() -> np.ndarray:
    """(1 - gaussian) self-suppression mask, [NP, NP] (matches reference)."""
    rat_s = np.float32(0.05)
    sr = np.float32(NPH) * rat_s
    ind_r = np.arange(NPH, dtype=np.float32).reshape(1, NPH, 1)
    ind_c = np.arange(NPH, dtype=np.float32).reshape(1, 1, NPH)
    cent = np.arange(NPH, dtype=np.float32)
    cent_r = np.repeat(cent, NPH).reshape(NP, 1, 1)
    cent_c = np.tile(cent, NPH).reshape(NP, 1, 1)
    g = np.exp(-((ind_r - cent_r) ** 2) / (2.0 * sr * sr)) * np.exp(
        -((ind_c - cent_c) ** 2) / (2.0 * sr * sr)
    )
    return (1.0 - g).reshape(NP, NP)


def build_nc():
    nc = bacc.Bacc(trn_type="TRN2")

    x_d = nc.dram_tensor("x", [B_LOC, C, H * W], F32, kind="ExternalInput")
    mask_d = nc.dram_tensor("mask", [NP, NP], F16, kind="ExternalInput")
    alpha_d = nc.dram_tensor("alpha", [128, 1], F32, kind="ExternalInput")
    out_d = nc.dram_tensor("out", [B_LOC, 3, NP, TOPK], F32, kind="ExternalOutput")

    with TileContext(nc) as tc:
        with tc.tile_pool(name="const", bufs=1) as cpool, \
             tc.tile_pool(name="slab", bufs=4) as slabp, \
             tc.tile_pool(name="eslab", bufs=4) as epool, \
             tc.tile_pool(name="z", bufs=1) as zpool, \
             tc.tile_pool(name="work", bufs=3) as work, \
             tc.tile_pool(name="small", bufs=3) as small, \
             tc.tile_pool(name="ps", bufs=3, space="PSUM") as ps, \
             tc.tile_pool(name="psn", bufs=2, space="PSUM") as psn, \
             tc.tile_pool(name="dsc", bufs=4, space="DRAM") as dsc:

            # ---- constants
            mask_sb = cpool.tile([128, NCHUNK, NP], F16)  # mask[128i+p, m] at [p,i,m]
            nc.sync.dma_start(
                mask_sb, mask_d[:, :].rearrange("(i p) m -> p i m", p=128)
            )
            ones_k = cpool.tile([128, 1], F32)  # column-sum matmul lhsT
            nc.vector.memset(ones_k, 1.0)
            av = cpool.tile([128, 1], F32)  # alpha
            nc.sync.dma_start(av, alpha_d[:, :])
            av_h = cpool.tile([128, 1], F32)  # alpha/2
            nc.vector.tensor_scalar_mul(av_h, av, 0.5)
            scale_E = [av, av, av_h]
            c15 = cpool.tile([128, 8], F32)
            nc.vector.memset(c15, 1.5)
            chalf = cpool.tile([128, 8], F32)
            nc.vector.memset(chalf, 0.5)
            cone_i = cpool.tile([128, 8], I32)
            nc.vector.memset(cone_i, 1)
            cmagic = cpool.tile([128, 8], I32)
            nc.vector.memset(cmagic, 0x5F3759DF)

            def emit_rsqrt(src, out_dtype, tag, iters):
                """1/sqrt(src) on [128,8]: DVE bit-hack seed + gpsimd Newton."""
                yi = small.tile([128, 8], I32, name="yi", tag=f"yi{tag}")
                nc.vector.tensor_tensor(out=yi, in0=src.bitcast(I32),
                                        in1=cone_i, op=ALU.logical_shift_right)
                y0i = small.tile([128, 8], I32, name="y0i", tag=f"y0{tag}")
                nc.vector.tensor_tensor(out=y0i, in0=cmagic, in1=yi,
                                        op=ALU.subtract)
                cur = y0i.bitcast(F32)
                for it in range(iters):
                    t1 = small.tile([128, 8], F32, name="t1", tag=f"t1{tag}")
                    nc.gpsimd.tensor_tensor(out=t1, in0=cur, in1=cur,
                                            op=ALU.mult)
                    t2 = small.tile([128, 8], F32, name="t2", tag=f"t2{tag}")
                    nc.gpsimd.tensor_tensor(out=t2, in0=t1, in1=src,
                                            op=ALU.mult)
                    t2h = small.tile([128, 8], F32, name="t2h", tag=f"t2h{tag}")
                    nc.gpsimd.tensor_tensor(out=t2h, in0=t2, in1=chalf,
                                            op=ALU.mult)
                    t3 = small.tile([128, 8], F32, name="t3", tag=f"t3{tag}")
                    nc.gpsimd.tensor_tensor(out=t3, in0=c15, in1=t2h,
                                            op=ALU.subtract)
                    odt = out_dtype if it == iters - 1 else F32
                    t4 = small.tile([128, 8], odt, name="t4", tag=f"t4{tag}{it}")
                    nc.gpsimd.tensor_tensor(out=t4, in0=cur, in1=t3,
                                            op=ALU.mult)
                    cur = t4
                return cur

            # ---- phase N: channel norms + normalized z (per batch element).
            zp = {}

            def emit_norm(b):
                xs = slabp.tile([128, H * W], F32, name=f"xs{b}", tag="slab16")
                nc.sync.dma_start(xs, x_d[b])
                xr = xs.rearrange("c (i r j s) -> c r s i j", r=PS, s=PS, j=NPH)
                for p in range(PS):
                    zv = xr[:, p, p]
                    zsq = work.tile([128, NP], F32, name="zsq", tag="zsq",
                                    bufs=2)
                    nc.scalar.activation(
                        zsq.rearrange("c (a b) -> c a b", a=NPH), zv, AF.Square)
                    nrmT = psn.tile([128, NCHUNK], F32, name="nrmT", tag="nrmT")
                    for i in range(NCHUNK):
                        nc.tensor.matmul(nrmT[:, i:i + 1],
                                         zsq[:, 128 * i:128 * (i + 1)],
                                         ones_k, start=True, stop=True)
                    rTn = small.tile([128, NCHUNK], F32, name="rTn", tag="rTn")
                    nc.vector.tensor_copy(rTn, nrmT)
                    inv = emit_rsqrt(rTn, F32, "n", iters=2)
                    inv_dram = dsc.tile([NP], F32, name="inv_dram",
                                        tag="inv_dram")
                    nc.sync.dma_start(
                        inv_dram[:].rearrange("(i p) -> p i", p=128), inv)
                    ibc = work.tile([128, NP], F32, name="ibc", tag="ibc",
                                    bufs=2)
                    nc.sync.dma_start(
                        ibc,
                        inv_dram[:].rearrange("(a m) -> a m", a=1)
                        .partition_broadcast(128))
                    z = zpool.tile([128, NP], F32R, name=f"z{b}{p}",
                                   tag=f"z{b}{p}", bufs=1)
                    nc.gpsimd.tensor_tensor(
                        out=z.rearrange("c (a b) -> c a b", a=NPH), in0=zv,
                        in1=ibc.rearrange("c (a b) -> c a b", a=NPH),
                        op=ALU.mult)
                    zp[(b, p)] = z

            # ---- phase M
            def emit_E_chunks(st, lo, hi):
                """chunks [lo,hi) of the merged q0/q1/q2 E-phase of batch b."""
                b = st["b"]
                for i in range(lo, hi):
                    for q in range(2):
                        G = ps.tile([128, NP], F32, name="G", tag="G")
                        z = zp[(b, q)]
                        for h in range(2):
                            nc.tensor.matmul(
                                G[:, 512 * h:512 * (h + 1)],
                                z[:, 128 * i:128 * (i + 1)],
                                z[:, 512 * h:512 * (h + 1)],
                                start=True, stop=True)
                        nc.vector.scalar_tensor_tensor(
                            out=st["s"][q][:, i, :], in0=G, scalar=1.0,
                            in1=mask_sb[:, i, :], op0=ALU.mult, op1=ALU.mult)
                        nc.scalar.activation(
                            st["e"][q][:, i, :], st["s"][q][:, i, :],
                            AF.Exp, scale=scale_E[q],
                            accum_out=st["rT"][q][:, i:i + 1])
                    s2 = work.tile([128, NP], F16, name="s2", tag="s2", bufs=2)
                    nc.gpsimd.tensor_tensor(out=s2, in0=st["s"][0][:, i, :],
                                            in1=st["s"][1][:, i, :], op=ALU.add)
                    st["s2scr"][i] = s2
                    if i > 0:
                        j = i - 1
                        nc.scalar.activation(
                            st["e"][2][:, j, :], st["s2scr"][j],
                            AF.Exp, scale=scale_E[2],
                            accum_out=st["rT"][2][:, j:j + 1])

            def finish_u(st, q):
                uT = small.tile([128, NCHUNK], F32, name="uT", tag=f"uT{q}")
                nc.vector.reciprocal(uT, st["rT"][q])
                st["uT"][q] = uT
                rsq = emit_rsqrt(st["rT"][q], F16, "u", iters=1)
                u_dram = dsc.tile([NP], F16, name="u_dram", tag="u_dram")
                nc.sync.dma_start(
                    u_dram[:].rearrange("(i p) -> p i", p=128), rsq)
                squbc = work.tile([128, NP], F16, name="squbc",
                                  tag="squbc", bufs=3)
                nc.sync.dma_start(
                    squbc,
                    u_dram[:].rearrange("(a m) -> a m", a=1)
                    .partition_broadcast(128))
                st["squbc"][q] = squbc

            def emit_E_finish(st):
                finish_u(st, 0)
                finish_u(st, 1)
                nc.scalar.activation(
                    st["e"][2][:, NCHUNK - 1, :], st["s2scr"][NCHUNK - 1],
                    AF.Exp, scale=scale_E[2],
                    accum_out=st["rT"][2][:, NCHUNK - 1:NCHUNK])
                finish_u(st, 2)

            def new_stage(b):
                st = dict(b=b, s={}, e={}, rT={}, uT={}, squbc={}, s2scr={})
                for q in range(2):
                    st["s"][q] = slabp.tile([128, NCHUNK, NP], F16,
                                            name=f"s{q}", tag="slab16")
                for q in range(3):
                    st["e"][q] = epool.tile([128, NCHUNK, NP], F16,
                                            name=f"e{q}", tag="eslab")
                    st["rT"][q] = small.tile([128, NCHUNK], F32, name="rT",
                                             tag=f"rT{q}")
                return st

            def emit_F(st, q):
                """v = E*sqrt(u[m]) -> top-8 -> (t3^2)*u[n] -> store."""
                b = st["b"]
                t8s = work.tile([128, NCHUNK, 8], F16, name="t8s", tag="t8s",
                                bufs=2)
                for i in range(NCHUNK):
                    v = work.tile([128, NP], F16, name="v", tag="v")
                    nc.vector.tensor_tensor(out=v, in0=st["e"][q][:, i, :],
                                            in1=st["squbc"][q], op=ALU.mult)
                    nc.vector.max(out=t8s[:, i, :], in_=v)
                w = work.tile([128, NCHUNK, TOPK], F32, name="w", tag="w",
                              bufs=2)
                u3 = st["uT"][q].unsqueeze(-1).to_broadcast(
                    [128, NCHUNK, TOPK])
                nc.vector.tensor_tensor(out=w, in0=t8s[:, :, :TOPK], in1=u3,
                                        op=ALU.mult)
                oacc = work.tile([128, NCHUNK, TOPK], F32, name="oacc",
                                 tag="oacc", bufs=2)
                nc.vector.tensor_tensor(out=oacc, in0=w, in1=t8s[:, :, :TOPK],
                                        op=ALU.mult)
                dst = out_d[b, q].rearrange("(i p) k -> p i k", p=128)
                nc.sync.dma_start(dst, oacc)

            # ---- emission schedule (software pipeline across the 2 batches)
            emit_norm(0)
            emit_norm(1)
            st0 = new_stage(0)
            emit_E_chunks(st0, 0, NCHUNK)
            emit_E_finish(st0)
            st1 = new_stage(1)
            emit_F(st0, 0)
            emit_E_chunks(st1, 0, 4)
            emit_F(st0, 1)
            emit_E_chunks(st1, 4, NCHUNK)
            emit_E_finish(st1)
            emit_F(st0, 2)
            emit_F(st1, 0)
            emit_F(st1, 1)
            emit_F(st1, 2)

    nc.compile()
    return nc


_NC_CACHE = None


def _get_nc():
    global _NC_CACHE
    if _NC_CACHE is None:
        _NC_CACHE = build_nc()
    return _NC_CACHE


def kernel(x: np.ndarray, alpha: np.ndarray) -> np.ndarray:
    global LAST_EXEC_NS
    x = np.ascontiguousarray(np.asarray(x, dtype=np.float32))
    alpha_arr = np.full((128, 1), np.float32(np.asarray(alpha)),
                        dtype=np.float32)
    mask = _build_mask().astype(np.float16)

    nc = _get_nc()
    in_maps = []
    for core in range(N_CORES):
        xs = x[core * B_LOC:(core + 1) * B_LOC].reshape(B_LOC, C, H * W)
        in_maps.append({"x": np.ascontiguousarray(xs), "mask": mask,
                        "alpha": alpha_arr})
    res = run_bass_kernel_spmd(nc, in_maps, core_ids=list(range(N_CORES)))
    LAST_EXEC_NS = res.exec_time_ns

    # assemble: out[bg, k, 2i+dr, 2j+dc] from T_q[b, n=i*32+j, k]
    out = np.empty((B_FULL, TOPK, H, W), dtype=np.float32)
    for core in range(N_CORES):
        t = res.results[core]["out"]  # [B_LOC, 3, NP, TOPK]
        for bl in range(B_LOC):
            bg = core * B_LOC + bl
            tq = t[bl].reshape(3, NPH, NPH, TOPK).transpose(0, 3, 1, 2)
            out[bg, :, 0::2, 0::2] = tq[0]
            out[bg, :, 1::2, 1::2] = tq[1]
            out[bg, :, 0::2, 1::2] = tq[2]
            out[bg, :, 1::2, 0::2] = tq[2]
    return out
